# revision 62
# baseline (speedup 1.0000x reference)
"""Trainium2 Bass kernel for nn_Net_int_12421045420311 (GNN message passing).

Model (see problem reference):
  out = relu(x @ Wn + bn)                         [N, 64]
  ea  = relu(edge_attr @ We + be)                 [E, 12]
  Wedge = (relu(ea @ W1 + b1) @ W2 + b2)          [E, 64, 64]
  3x: msg_e = out[src_e] @ Wedge_e ; agg = scatter_mean(msg, dst) ;
      h = GRU(relu(agg + conv_b), h) ; out = h
  pair readout over 65536 node pairs + train-mode batchnorm + linear head.

Distribution over 8 cores: edges sorted by dst and sharded by dst range
(N/8 nodes per core) so every core owns complete scatter sums for its
nodes; the node table is AllGathered (bf16) between iterations; pairs
are sharded for the readout and batchnorm stats are AllReduced.

Device techniques (v2):
 - scatter-mean one-hots (with 1/deg folded in) precomputed on HOST as
   bf16 and kept resident in SBUF -- no per-iteration GPSIMD build.
 - Wedge stored TRANSPOSED per edge ([o-major, i-minor]) with the edge
   MLP bias b2 folded in (K=1 ones-row matmul in phase A), so the
   per-edge contraction msg_e = sum_i a_e[i] * W_e[:, i] runs as a
   dense bf16 2x-mode tensor_tensor + a 2-level 2x add tree + a 16-way
   dense reduce on DVE.
 - phase A (per-edge weight matrices) fused into iteration 0: each
   Wedge chunk is consumed from SBUF and simultaneously written to
   DRAM (bf16) for iterations 1-2 to stream back.
 - node tables bf16 end-to-end (gathers, AllGathers); GRU state f32.
 - gathers via the GPSIMD dma_gather extended instruction.
"""

from contextlib import ExitStack

import numpy as np
import ml_dtypes

import concourse.bass as bass
import concourse.mybir as mybir
import concourse.tile as tile
import bass_rust as _bass_rust
from concourse import bass_utils
from concourse.bass import ts, ds
from concourse.library_config import all_libraries, standard
from concourse.masks import make_identity

F32 = mybir.dt.float32
BF16 = mybir.dt.bfloat16
I16 = mybir.dt.int16
AF = mybir.ActivationFunctionType
OP = mybir.AluOpType

N_CORES = 8
DIM = 64
GATHER_PIECE = 1024   # max idxs per dma_gather (descriptor-ring capacity)
HID = 128
EDIM = 12
EPS = 1e-5
TILE = 512            # nodes per scatter tile


class Cfg:
    def __init__(self, n_nodes, n_edges, n_pairs, ch_per_tile, p_loc,
                 n_iters=3, readout=True):
        self.n = n_nodes
        self.e = n_edges
        self.p = n_pairs
        self.n_loc = n_nodes // N_CORES
        self.p_loc = p_loc          # padded pairs per core (idx30-sharded)
        self.tile_nodes = min(TILE, self.n_loc)
        self.n_tiles = self.n_loc // self.tile_nodes
        self.subt = self.tile_nodes // 128
        self.ch_per_tile = ch_per_tile
        self.chunks = self.n_tiles * ch_per_tile
        self.e_pad = self.chunks * 128
        self.p_chunks = self.p_loc // 128
        self.n_iters = n_iters
        self.readout = readout
        self.key = (n_nodes, n_edges, n_pairs, ch_per_tile, p_loc,
                    n_iters, readout)


def _ap(base, dims, off=0):
    return bass.AP(base.tensor, base.offset + off, [list(d) for d in dims])


def _legalize_waits(nc, keep=1, keep_extended=0):
    """Split multi-wait instructions into preceding single-wait NoOps.

    This walrus build's setupSyncWait accepts at most one sync wait per
    instruction, while Tile attaches one wait per producer semaphore.
    Waits execute on the engine sequencer in program order, so hoisting
    them onto NoOps preserves semantics.
    """
    n = 0
    ext = ("DMAGatherAnt", "DMAScatterAddAnt", "KVWritebackAnt",
           "PagedWritebackAnt")
    for f in nc.m.functions:
        for bb in f.blocks:
            out = []
            for ins in bb.instructions:
                si = ins.sync_info
                k = keep_extended if type(ins).__name__.removeprefix("Inst") in ext else keep
                if si is not None and si.on_wait is not None and len(si.on_wait) > k:
                    waits = list(si.on_wait)
                    for w in (waits[:-k] if k else waits):
                        nop = mybir.InstNoOp(name=f"WS-{n}", text_hint="waitsplit")
                        n += 1
                        nop.engine = ins.engine
                        nop.sync_info = mybir.SyncInfo(on_wait=[w], on_update=[])
                        nc.register_instruction(nop, overwrite=True)
                        out.append(nop)
                    ins.sync_info = mybir.SyncInfo(
                        on_wait=(waits[-k:] if k else []),
                        on_update=list(si.on_update))
                out.append(ins)
            bb.instructions = out
    return n


def _insert_library_loads(nc):
    """bacc.insert_library_loads equivalent for plain Bass: dma_gather &
    friends need the 'mlp' GPSIMD ucode library loaded."""
    mask = {}
    for lib in all_libraries:
        for t in lib.instructions:
            mask[t] = mask.get(t, 0) | (1 << lib.index)
    _bass_rust.insert_library_loads(nc, mask, len(all_libraries), standard.index)


def _gather_split(nc, out_tile, table, idx_sb, total):
    """dma_gather in <=GATHER_PIECE chunks (descriptor carveout is ~1024
    descs; one big gather would deadlock awaiting ring space)."""
    for off in range(0, total, GATHER_PIECE):
        w = min(GATHER_PIECE, total - off)
        nc.gpsimd.dma_gather(
            out_tile[:, off // 128:(off + w) // 128, :], table[:],
            idx_sb[:, off // 16:(off + w) // 16], w, w, DIM)


def build_nc(c: Cfg):
    nc = bass.Bass()
    WSQ = DIM * DIM
    G3 = 3 * DIM

    # ---------------- I/O ----------------
    xTl = nc.dram_tensor("xTl", [9, c.n_loc], F32, kind="ExternalInput")
    Wn_ext = nc.dram_tensor("Wn_ext", [9, DIM], F32, kind="ExternalInput")
    eaT = nc.dram_tensor("eaT", [19, c.e_pad], F32, kind="ExternalInput")
    We_in = nc.dram_tensor("We_in", [19, EDIM], F32, kind="ExternalInput")
    W1_in = nc.dram_tensor("W1_in", [EDIM, HID], F32, kind="ExternalInput")
    be_c = nc.dram_tensor("be_c", [EDIM, 1], F32, kind="ExternalInput")
    b1_c = nc.dram_tensor("b1_c", [HID, 1], F32, kind="ExternalInput")
    W2p_in = nc.dram_tensor("W2p_in", [HID, WSQ], BF16, kind="ExternalInput")
    b2r_in = nc.dram_tensor("b2r_in", [DIM, DIM], F32, kind="ExternalInput")
    ohp = nc.dram_tensor("ohp", [128, c.chunks * TILE], BF16,
                         kind="ExternalInput")
    conv_bc = nc.dram_tensor("conv_bc", [128, DIM], F32, kind="ExternalInput")
    # per-tile chunk 0 holds only core-local-src edges (gathered from
    # cc_in, overlapping the AllGather); the rest gather from cc_out
    gidxl = nc.dram_tensor("gidxl", [128, c.n_tiles * 8], I16,
                           kind="ExternalInput")
    gidxg = nc.dram_tensor(
        "gidxg", [128, c.n_tiles * (c.ch_per_tile - 1) * 8], I16,
        kind="ExternalInput")
    WihT = nc.dram_tensor("WihT", [DIM, 3 * DIM], BF16, kind="ExternalInput")
    WhhT = nc.dram_tensor("WhhT", [DIM, 3 * DIM], BF16, kind="ExternalInput")
    b_r = nc.dram_tensor("b_r", [DIM, 1], F32, kind="ExternalInput")
    b_z = nc.dram_tensor("b_z", [DIM, 1], F32, kind="ExternalInput")
    bihn = nc.dram_tensor("bihn", [DIM, 1], F32, kind="ExternalInput")
    bhhn = nc.dram_tensor("bhhn", [DIM, 1], F32, kind="ExternalInput")
    idx30 = nc.dram_tensor("idx30", [128, c.p_loc // 16], I16, kind="ExternalInput")
    idx31 = nc.dram_tensor("idx31", [128, c.p_loc // 16], I16, kind="ExternalInput")
    pmask_in = nc.dram_tensor("pmask_in", [128, c.p_chunks], BF16,
                              kind="ExternalInput")
    ea3T = nc.dram_tensor("ea3T", [8, c.p_loc], BF16, kind="ExternalInput")
    Wlw = nc.dram_tensor("Wlw", [8, G3], F32, kind="ExternalInput")
    Wlb = nc.dram_tensor("Wlb", [8, 1], F32, kind="ExternalInput")
    gamma = nc.dram_tensor("gamma", [1, G3], F32, kind="ExternalInput")
    beta = nc.dram_tensor("beta", [1, G3], F32, kind="ExternalInput")
    epsv = nc.dram_tensor("epsv", [1, G3], F32, kind="ExternalInput")
    y = nc.dram_tensor("y", [128, c.p_chunks], F32, kind="ExternalOutput")

    # internal DRAM. Node tables are [h | h@b2r] bf16 (256B rows: the min
    # dma_gather granule) -- the b2r half supplies the edge-MLP bias term
    # of the NNConv directly through the scatter matmul.
    wedge_d = nc.dram_tensor("wedge_d", [c.e_pad, WSQ], BF16)
    cc_in = [nc.dram_tensor(f"cc_in{i}", [c.n_loc, 2 * DIM], BF16)
             for i in range(c.n_iters + 1)]
    cc_out = [nc.dram_tensor(f"cc_out{i}", [c.n, 2 * DIM], BF16,
                             addr_space="Shared")
              for i in range(c.n_iters + 1)]
    st_in = nc.dram_tensor("st_in", [1, 2 * G3], F32)
    st_out = nc.dram_tensor("st_out", [1, 2 * G3], F32, addr_space="Shared")

    rgroups = [list(range(N_CORES))]

    with tile.TileContext(nc) as tc:
      with (
          tc.tile_pool(name="persist", bufs=1) as pp,
      ):
        # ------------- persistent small tensors -------------
        ident = pp.tile([128, 128], F32)
        make_identity(nc, ident[:])

        Wn_sb = pp.tile([9, DIM], F32)
        nc.sync.dma_start(Wn_sb[:], Wn_ext[:])
        convb_sb = pp.tile([128, DIM], F32)
        nc.sync.dma_start(convb_sb[:], conv_bc[:])
        b2r_sb = pp.tile([DIM, DIM], F32)
        nc.sync.dma_start(b2r_sb[:], b2r_in[:])
        WihT_sb = pp.tile([DIM, 3 * DIM], BF16)
        nc.sync.dma_start(WihT_sb[:], WihT[:])
        WhhT_sb = pp.tile([DIM, 3 * DIM], BF16)
        nc.sync.dma_start(WhhT_sb[:], WhhT[:])
        br_sb = pp.tile([DIM, 1], F32)
        nc.sync.dma_start(br_sb[:], b_r[:])
        bz_sb = pp.tile([DIM, 1], F32)
        nc.sync.dma_start(bz_sb[:], b_z[:])
        bihn_sb = pp.tile([DIM, 1], F32)
        nc.sync.dma_start(bihn_sb[:], bihn[:])
        bhhn_sb = pp.tile([DIM, 1], F32)
        nc.sync.dma_start(bhhn_sb[:], bhhn[:])
        gidxl_sb = pp.tile([128, c.n_tiles * 8], I16)
        gidxg_sb = pp.tile([128, c.n_tiles * (c.ch_per_tile - 1) * 8], I16)
        oh_sb = pp.tile([128, c.chunks, TILE], BF16)

        # wedge-source tensors live only through iteration 0
        wstk = ExitStack()
        wdp = wstk.enter_context(tc.tile_pool(name="wdat", bufs=1))
        W2p_sb = wdp.tile([HID, WSQ], BF16)
        h1T = wdp.tile([HID, c.e_pad], BF16)

        hT = pp.tile([DIM, c.n_loc], F32)       # node state (transposed)
        hTb = pp.tile([DIM, c.n_loc], BF16)     # bf16 copy for GRU matmuls

        def emit_rows_ag(idx, psT, rows):
            """hT -> [h | h@b2r] bf16 rows -> cc_in[idx] -> AllGather."""
            for g in range(c.n_loc // 128):
                ptg = psT.tile([128, 128], F32, tag="ptx")
                nc.tensor.transpose(ptg[:, :DIM], hT[:, ts(g, 128)],
                                    ident[:DIM, :DIM])
                nc.scalar.copy(rows[:, g, 0:DIM], ptg[:, :DIM])
                pb2 = psT.tile([128, DIM], F32, tag="pb2")
                nc.tensor.matmul(pb2[:], hT[:, ts(g, 128)], b2r_sb[:],
                                 start=True, stop=True)
                nc.scalar.copy(rows[:, g, DIM:2 * DIM], pb2[:])
            # rows stored partition-major (row = p*NG + g): contiguous DMA;
            # all gather indices are host-remapped to match.
            nc.sync.dma_start(
                cc_in[idx].rearrange("(p g) d -> p g d", p=128), rows[:])
            nc.gpsimd.collective_compute(
                "AllGather", OP.bypass, replica_groups=rgroups,
                ins=[cc_in[idx].ap().opt()], outs=[cc_out[idx].ap().opt()])

        # ------------- phase 0: initial node embeddings -------------
        # h0T = relu(Wn_ext.T @ x_extT_local); table built via AllGather
        with (
            tc.tile_pool(name="p0", bufs=2) as sp,
            tc.tile_pool(name="p0ps", bufs=2, space="PSUM") as ps2,
        ):
            xl = sp.tile([9, c.n_loc], F32, tag="xl")
            nc.sync.dma_start(xl[:], xTl[:])
            for j in range(0, c.n_loc, 512):
                w = min(512, c.n_loc - j)
                ph = ps2.tile([DIM, 512], F32, tag="p0h")
                nc.tensor.matmul(ph[:, :w], Wn_sb[:], xl[:, ds(j, w)],
                                 start=True, stop=True)
                nc.scalar.activation(hT[:, ds(j, w)], ph[:, :w], AF.Relu)
            rows0 = sp.tile([128, c.n_loc // 128, 2 * DIM], BF16, tag="r0")
            emit_rows_ag(0, ps2, rows0)

        # bulk loads issued after the phase-0 critical path
        nc.sync.dma_start(gidxl_sb[:], gidxl[:])
        nc.sync.dma_start(gidxg_sb[:], gidxg[:])
        nc.sync.dma_start(
            oh_sb[:], ohp.rearrange("p (c t) -> p c t", c=c.chunks))
        nc.sync.dma_start(W2p_sb[:], W2p_in[:])

        # ------------- phase A prep: edge MLP -> h1T (bf16) -------------
        with (
            tc.tile_pool(name="pa", bufs=3) as sp,
            tc.tile_pool(name="pah", bufs=1) as hp,
            tc.tile_pool(name="paps", bufs=2, space="PSUM") as psA,
        ):
            W1_sb = hp.tile([EDIM, HID], F32)
            nc.sync.dma_start(W1_sb[:], W1_in[:])
            We_sb = hp.tile([19, EDIM], F32)
            nc.sync.dma_start(We_sb[:], We_in[:])
            be_sb = hp.tile([EDIM, 1], F32)
            nc.sync.dma_start(be_sb[:], be_c[:])
            b1_sb = hp.tile([HID, 1], F32)
            nc.sync.dma_start(b1_sb[:], b1_c[:])
            ea_sb = hp.tile([EDIM, c.e_pad], F32)
            for j in range(0, c.e_pad, 512):
                w = min(512, c.e_pad - j)
                et = sp.tile([19, 512], F32, tag="et")
                nc.sync.dma_start(et[:, :w], eaT[:, ds(j, w)])
                pe = psA.tile([EDIM, 512], F32, tag="pe")
                nc.tensor.matmul(pe[:, :w], We_sb[:], et[:, :w],
                                 start=True, stop=True)
                nc.scalar.activation(ea_sb[:, ds(j, w)], pe[:, :w],
                                     AF.Relu, bias=be_sb[:])
            for j in range(0, c.e_pad, 512):
                w = min(512, c.e_pad - j)
                ph1 = psA.tile([HID, 512], F32, tag="ph1")
                nc.tensor.matmul(ph1[:, :w], W1_sb[:], ea_sb[:, ds(j, w)],
                                 start=True, stop=True)
                nc.scalar.activation(h1T[:, ds(j, w)], ph1[:, :w], AF.Relu,
                                     bias=b1_sb[:])

        # ------------- 3 message-passing iterations -------------
        # iteration 0 also computes Wedge (transposed, +b2) chunk by chunk,
        # consuming each chunk from SBUF and writing it to DRAM for the
        # later iterations to stream back.
        for it in range(c.n_iters):
            tab_prev = cc_out[it]
            with (
                tc.tile_pool(name=f"it{it}", bufs=1) as ip,
                tc.tile_pool(name=f"itw{it}", bufs=2) as wp,
                tc.tile_pool(name=f"itv{it}", bufs=1) as vp,
                tc.tile_pool(name=f"its{it}", bufs=2) as sp,
                tc.tile_pool(name=f"itpt{it}", bufs=1, space="PSUM") as psT,
                tc.tile_pool(name=f"itpm{it}", bufs=2, space="PSUM") as psM,
            ):
                nc.scalar.copy(hTb[:], hT[:])
                # gathered rows [a | a@b2r]; msgs later overwrite the a half
                A_sb = ip.tile([128, c.chunks, 2 * DIM], BF16)
                CH = c.ch_per_tile
                for t in range(c.n_tiles):
                    nc.gpsimd.dma_gather(
                        A_sb[:, t * CH:t * CH + 1, :], cc_in[it][:],
                        gidxl_sb[:, t * 8:(t + 1) * 8], 128, 128, 2 * DIM)
                rows_pt = (CH - 1) * 128
                for t in range(c.n_tiles):
                    for off in range(0, rows_pt, GATHER_PIECE):
                        w = min(GATHER_PIECE, rows_pt - off)
                        c0 = t * CH + 1 + off // 128
                        nc.gpsimd.dma_gather(
                            A_sb[:, c0:c0 + w // 128, :], tab_prev[:],
                            gidxg_sb[:, (t * rows_pt + off) // 16:
                                     (t * rows_pt + off + w) // 16],
                            w, w, 2 * DIM)
                mT = ip.tile([DIM, c.n_loc], BF16)

                with tc.tile_pool(name=f"itpw{it}", bufs=2,
                                  space="PSUM") as psW:
                    def do_chunks(ch, nch):
                        # process nch (1 or 2) adjacent edge chunks in one
                        # DVE chain to amortize per-instruction overhead
                        NW = nch * WSQ
                        wt = wp.tile([128, 2, WSQ], BF16, tag="wt")
                        if it == 0:
                            # WedgeT chunks = h1 @ W2p, bf16
                            for j in range(nch):
                                for q in range(4):
                                    pwq = psW.tile([128, WSQ // 4], F32,
                                                   tag="pw")
                                    for hh in range(2):
                                        nc.tensor.matmul(
                                            pwq[:, ts(hh, 512)],
                                            h1T[:, ts(ch + j, 128)],
                                            W2p_sb[:, ds(q * 1024 + hh * 512,
                                                         512)],
                                            start=True, stop=True)
                                    nc.scalar.copy(
                                        wt[:, j, ds(q * 1024, 1024)], pwq[:])
                            nc.sync.dma_start(
                                _ap(wedge_d.ap(),
                                    [[WSQ, 128], [128 * WSQ, nch], [1, WSQ]],
                                    off=ch * 128 * WSQ),
                                wt[:, :nch, :])
                        else:
                            nc.sync.dma_start(
                                wt[:, :nch, :],
                                _ap(wedge_d.ap(),
                                    [[WSQ, 128], [128 * WSQ, nch], [1, WSQ]],
                                    off=ch * 128 * WSQ))
                        # msg_e[o] = sum_i a_e[i] * WT_e[o, i] on DVE
                        AD = c.chunks * 2 * DIM
                        prod = vp.tile([128, 2, WSQ], BF16, tag="prod")
                        nc.vector.tensor_tensor(
                            _ap(prod[:], [[2 * WSQ, 128], [WSQ, nch],
                                          [DIM, DIM], [1, DIM]]),
                            _ap(wt[:], [[2 * WSQ, 128], [WSQ, nch],
                                        [DIM, DIM], [1, DIM]]),
                            _ap(A_sb[:], [[AD, 128], [2 * DIM, nch],
                                          [0, DIM], [1, DIM]],
                                off=ch * 2 * DIM),
                            OP.mult)
                        pr1 = vp.tile([128, 2, WSQ // 2], BF16, tag="pr1")
                        nc.vector.tensor_tensor(
                            _ap(pr1[:], [[WSQ, 128], [2048, nch],
                                         [32, DIM], [1, 32]]),
                            _ap(prod[:], [[2 * WSQ, 128], [WSQ, nch],
                                          [DIM, DIM], [1, 32]]),
                            _ap(prod[:], [[2 * WSQ, 128], [WSQ, nch],
                                          [DIM, DIM], [1, 32]], off=32),
                            OP.add)
                        pr2 = vp.tile([128, 2, WSQ // 4], BF16, tag="pr2")
                        nc.vector.tensor_tensor(
                            _ap(pr2[:], [[WSQ // 2, 128], [1024, nch],
                                         [16, DIM], [1, 16]]),
                            _ap(pr1[:], [[WSQ, 128], [2048, nch],
                                         [32, DIM], [1, 16]]),
                            _ap(pr1[:], [[WSQ, 128], [2048, nch],
                                         [32, DIM], [1, 16]], off=16),
                            OP.add)
                        pr3 = vp.tile([128, 2, WSQ // 8], BF16, tag="pr3")
                        nc.vector.tensor_tensor(
                            _ap(pr3[:], [[WSQ // 4, 128], [512, nch],
                                         [8, DIM], [1, 8]]),
                            _ap(pr2[:], [[WSQ // 2, 128], [1024, nch],
                                         [16, DIM], [1, 8]]),
                            _ap(pr2[:], [[WSQ // 2, 128], [1024, nch],
                                         [16, DIM], [1, 8]], off=8),
                            OP.add)
                        with nc.allow_low_precision(
                                reason="bf16 msg sums ok at 2e-2 tol"):
                            nc.vector.tensor_reduce(
                                _ap(A_sb[:], [[AD, 128], [2 * DIM, nch],
                                              [1, DIM]], off=ch * 2 * DIM),
                                _ap(pr3[:], [[WSQ // 4, 128], [512, nch],
                                             [8, DIM], [1, 8]]),
                                mybir.AxisListType.X, OP.add)

                    # local-src chunks first: their gather needs no AllGather
                    for t in range(c.n_tiles):
                        do_chunks(t * CH, 1)
                    for t in range(c.n_tiles):
                        for k in range(1, CH, 2):
                            do_chunks(t * CH + k, min(2, CH - k))
                        # scatter-mean for tile t via one-hot matmuls; the
                        # moving operand [msgs | a@b2r] yields the message
                        # sum and the NNConv b2 term in one pass.
                        pm = psM.tile([128, c.subt * 2 * DIM], F32, tag="pm")
                        for s in range(c.subt):
                            for k in range(c.ch_per_tile):
                                ch = t * c.ch_per_tile + k
                                nc.tensor.matmul(
                                    pm[:, ts(s, 2 * DIM)],
                                    oh_sb[:, ch, ds(s * 128, 128)],
                                    A_sb[:, ch, :], start=(k == 0),
                                    stop=(k == c.ch_per_tile - 1))
                        btmp = sp.tile([128, c.subt * DIM], F32, tag="btmp")
                        nc.scalar.copy(
                            btmp[:],
                            _ap(pm[:], [[c.subt * 2 * DIM, 128],
                                        [2 * DIM, c.subt], [1, DIM]],
                                off=DIM))
                        mrow = sp.tile([128, c.subt * DIM], F32, tag="mrow")
                        nc.vector.tensor_tensor(
                            _ap(mrow[:], [[c.subt * DIM, 128], [DIM, c.subt],
                                          [1, DIM]]),
                            _ap(pm[:], [[c.subt * 2 * DIM, 128],
                                        [2 * DIM, c.subt], [1, DIM]]),
                            btmp[:], OP.add)
                        nc.vector.tensor_tensor(
                            mrow[:], mrow[:],
                            _ap(convb_sb[:], [[DIM, 128], [0, c.subt], [1, DIM]]),
                            OP.add)
                        nc.scalar.activation(mrow[:], mrow[:], AF.Relu)
                        for s in range(c.subt):
                            ptm = psT.tile([128, 128], F32, tag="ptx")
                            nc.tensor.transpose(ptm[:DIM, :], mrow[:, ts(s, DIM)],
                                                ident[:])
                            nc.scalar.copy(
                                mT[:, ds(t * c.tile_nodes + s * 128, 128)],
                                ptm[:DIM, :])

                # ---- GRU (transposed space), 512-node pieces ----
                with tc.tile_pool(name=f"itpg{it}", bufs=1,
                                  space="PSUM") as psG:
                    for j in range(0, c.n_loc, 512):
                        w = min(512, c.n_loc - j)
                        pr = psG.tile([DIM, 512], F32, tag="pr")
                        pz = psG.tile([DIM, 512], F32, tag="pz")
                        pxn = psG.tile([DIM, 512], F32, tag="pxn")
                        phn = psG.tile([DIM, 512], F32, tag="phn")
                        nc.tensor.matmul(pr[:, :w], WihT_sb[:, 0:DIM],
                                         mT[:, ds(j, w)], start=True, stop=False)
                        nc.tensor.matmul(pr[:, :w], WhhT_sb[:, 0:DIM],
                                         hTb[:, ds(j, w)], start=False, stop=True)
                        nc.tensor.matmul(pz[:, :w], WihT_sb[:, DIM:2 * DIM],
                                         mT[:, ds(j, w)], start=True, stop=False)
                        nc.tensor.matmul(pz[:, :w], WhhT_sb[:, DIM:2 * DIM],
                                         hTb[:, ds(j, w)], start=False, stop=True)
                        nc.tensor.matmul(pxn[:, :w], WihT_sb[:, 2 * DIM:],
                                         mT[:, ds(j, w)], start=True, stop=True)
                        nc.tensor.matmul(phn[:, :w], WhhT_sb[:, 2 * DIM:],
                                         hTb[:, ds(j, w)], start=True, stop=True)
                        r_sb = sp.tile([DIM, 512], F32, tag="r")
                        nc.scalar.activation(r_sb[:, :w], pr[:, :w], AF.Sigmoid,
                                             bias=br_sb[:])
                        z_sb = sp.tile([DIM, 512], F32, tag="z")
                        nc.scalar.activation(z_sb[:, :w], pz[:, :w], AF.Sigmoid,
                                             bias=bz_sb[:])
                        ghn = sp.tile([DIM, 512], F32, tag="ghn")
                        nc.scalar.activation(ghn[:, :w], phn[:, :w], AF.Identity,
                                             bias=bhhn_sb[:])
                        nc.vector.tensor_tensor(ghn[:, :w], r_sb[:, :w], ghn[:, :w],
                                                OP.mult)
                        s_sb = sp.tile([DIM, 512], F32, tag="s")
                        nc.vector.tensor_tensor(s_sb[:, :w], pxn[:, :w], ghn[:, :w],
                                                OP.add)
                        n_sb = sp.tile([DIM, 512], F32, tag="n")
                        nc.scalar.activation(n_sb[:, :w], s_sb[:, :w], AF.Tanh,
                                             bias=bihn_sb[:])
                        d_sb = sp.tile([DIM, 512], F32, tag="d")
                        nc.vector.tensor_tensor(d_sb[:, :w], hT[:, ds(j, w)],
                                                n_sb[:, :w], OP.subtract)
                        nc.vector.tensor_tensor(d_sb[:, :w], z_sb[:, :w], d_sb[:, :w],
                                                OP.mult)
                        nc.vector.tensor_tensor(hT[:, ds(j, w)], n_sb[:, :w],
                                                d_sb[:, :w], OP.add)

                # ---- rows of new h -> cc_in[it+1]; AllGather ----
                rows = ip.tile([128, c.n_loc // 128, 2 * DIM], BF16)
                emit_rows_ag(it + 1, psT, rows)
            if it == 0:
                wstk.close()    # free W2p/h1T SBUF for later phases

        # ------------- readout -------------
        if not c.readout:
            dbg = pp.tile([128, 8], F32)
            nc.sync.dma_start(dbg[:, :1], cc_out[c.n_iters][:128, :1])
            nc.vector.tensor_copy(dbg[:, 1:2], dbg[:, :1])
            yz = pp.tile([128, c.p_chunks], F32)
            nc.vector.memset(yz[:], 0.0)
            nc.vector.tensor_tensor(yz[:, :1], yz[:, :1], dbg[:, 1:2], OP.add)
            nc.sync.dma_start(y[:], yz[:])
        if c.readout:
          with (
              tc.tile_pool(name="ro", bufs=1) as rp,
              tc.tile_pool(name="ros", bufs=2) as sp,
              tc.tile_pool(name="rops", bufs=2, space="PSUM") as psR,
              tc.tile_pool(name="rop1", bufs=1, space="PSUM") as psS,
          ):
              tab_fin = cc_out[c.n_iters]
              GP = GATHER_PIECE
              t0 = rp.tile([128, c.p_chunks, 2 * DIM], BF16)
              i30 = rp.tile([128, c.p_loc // 16], I16)
              nc.sync.dma_start(i30[:], idx30[:])
              t1 = rp.tile([128, c.p_chunks, 2 * DIM], BF16)
              i31 = rp.tile([128, c.p_loc // 16], I16)
              nc.sync.dma_start(i31[:], idx31[:])
              ea3_sb = rp.tile([8, c.p_loc], BF16)
              nc.sync.dma_start(ea3_sb[:], ea3T[:])
              pmask = rp.tile([128, c.p_chunks], BF16)
              nc.sync.dma_start(pmask[:], pmask_in[:])

              PIT = G3 + 1
              yh = rp.tile([128, c.p_chunks, PIT], BF16)  # [sum|prod|diff2|1]
              nc.vector.memset(yh[:, :, G3:G3 + 1], 1.0)
              pst = psS.tile([1, G3], F32, tag="ps_s")
              psq = psS.tile([1, G3], F32, tag="ps_q")
              # pairs are sharded by idx30's owner core, so t0 gathers hit
              # the core-local final rows (cc_in) and overlap the final
              # AllGather; only t1 needs the gathered table. yh and the
              # batchnorm stats are built per t1 piece.
              for off in range(0, c.p_loc, GP):
                  w = min(GP, c.p_loc - off)
                  nc.gpsimd.dma_gather(
                      t0[:, off // 128:(off + w) // 128, :],
                      cc_in[c.n_iters][:],
                      i30[:, off // 16:(off + w) // 16], w, w, 2 * DIM)
              for off in range(0, c.p_loc, GP):
                  w = min(GP, c.p_loc - off)
                  g0, gw = off // 128, w // 128
                  nc.gpsimd.dma_gather(
                      t1[:, g0:g0 + gw, :], tab_fin[:],
                      i31[:, off // 16:(off + w) // 16], w, w, 2 * DIM)
                  pcD = [[c.p_chunks * 2 * DIM, 128], [2 * DIM, gw], [1, DIM]]
                  yhD = [[c.p_chunks * PIT, 128], [PIT, gw], [1, DIM]]
                  pco = g0 * 2 * DIM
                  yho = g0 * PIT
                  nc.vector.tensor_tensor(
                      _ap(yh[:], yhD, off=yho), _ap(t0[:], pcD, off=pco),
                      _ap(t1[:], pcD, off=pco), OP.add)
                  nc.vector.tensor_tensor(
                      _ap(yh[:], yhD, off=yho + DIM), _ap(t0[:], pcD, off=pco),
                      _ap(t1[:], pcD, off=pco), OP.mult)
                  nc.vector.tensor_tensor(
                      _ap(yh[:], yhD, off=yho + 2 * DIM),
                      _ap(t0[:], pcD, off=pco),
                      _ap(t1[:], pcD, off=pco), OP.subtract)
                  nc.scalar.square(_ap(yh[:], yhD, off=yho + 2 * DIM),
                                   _ap(yh[:], yhD, off=yho + 2 * DIM))
                  for g in range(g0, g0 + gw):
                      ysq = sp.tile([128, G3], BF16, tag="ysq")
                      nc.scalar.square(
                          ysq[:],
                          _ap(yh[:], [[c.p_chunks * PIT, 128], [1, G3]],
                              off=g * PIT))
                      nc.tensor.matmul(
                          pst[:], pmask[:, ds(g, 1)],
                          _ap(yh[:], [[c.p_chunks * PIT, 128], [1, G3]],
                              off=g * PIT),
                          start=(g == 0), stop=(g == c.p_chunks - 1))
                      nc.tensor.matmul(
                          psq[:], pmask[:, ds(g, 1)], ysq[:],
                          start=(g == 0), stop=(g == c.p_chunks - 1))
              st_sb = sp.tile([1, 2 * G3], F32, tag="st")
              nc.vector.tensor_copy(st_sb[:, :G3], pst[:])
              nc.scalar.copy(st_sb[:, G3:], psq[:])
              nc.sync.dma_start(st_in[:], st_sb[:])
              nc.gpsimd.collective_compute(
                  "AllReduce", OP.add, replica_groups=rgroups,
                  ins=[st_in.ap().opt()], outs=[st_out.ap().opt()])
              stg = sp.tile([1, 2 * G3], F32, tag="stg")
              nc.sync.dma_start(stg[:], st_out[:])

              # mu, var, rstd (with one Newton step), g' = rstd*gamma,
              # b' = beta - mu*g'
              gam_sb = sp.tile([1, G3], F32, tag="gam")
              nc.sync.dma_start(gam_sb[:], gamma[:])
              bet_sb = sp.tile([1, G3], F32, tag="bet")
              nc.sync.dma_start(bet_sb[:], beta[:])
              mu = sp.tile([1, G3], F32, tag="mu")
              nc.vector.tensor_scalar_mul(mu[:], stg[:, :G3], 1.0 / c.p)
              var = sp.tile([1, G3], F32, tag="var")
              nc.vector.tensor_scalar_mul(var[:], stg[:, G3:], 1.0 / c.p)
              musq = sp.tile([1, G3], F32, tag="musq")
              nc.vector.tensor_tensor(musq[:], mu[:], mu[:], OP.mult)
              nc.vector.tensor_tensor(var[:], var[:], musq[:], OP.subtract)
              epsv_sb = sp.tile([1, G3], F32, tag="epsv")
              nc.sync.dma_start(epsv_sb[:], epsv[:])
              ve = sp.tile([1, G3], F32, tag="ve")
              nc.vector.tensor_tensor(ve[:], var[:], epsv_sb[:], OP.add)
              sq = sp.tile([1, G3], F32, tag="sq")
              nc.scalar.activation(sq[:], ve[:], AF.Sqrt)
              r0 = sp.tile([1, G3], F32, tag="r0")
              nc.vector.reciprocal(r0[:], sq[:])
              # Newton: r1 = r0*(1.5 - 0.5*(var+eps)*r0^2)
              t_ = sp.tile([1, G3], F32, tag="t_")
              nc.vector.tensor_tensor(t_[:], r0[:], r0[:], OP.mult)
              nc.vector.tensor_tensor(t_[:], t_[:], ve[:], OP.mult)
              nc.vector.tensor_scalar(t_[:], t_[:], -0.5, 1.5, OP.mult, OP.add)
              nc.vector.tensor_tensor(r0[:], r0[:], t_[:], OP.mult)
              gp = sp.tile([1, G3], F32, tag="gp")
              nc.vector.tensor_tensor(gp[:], r0[:], gam_sb[:], OP.mult)
              bp = sp.tile([1, G3], F32, tag="bp")
              nc.vector.tensor_tensor(bp[:], mu[:], gp[:], OP.mult)
              nc.vector.tensor_tensor(bp[:], bet_sb[:], bp[:], OP.subtract)

              gpb = sp.tile([8, G3], F32, tag="gpb")
              nc.gpsimd.partition_broadcast(gpb[:], gp[:])
              bpb = sp.tile([8, G3], F32, tag="bpb")
              nc.gpsimd.partition_broadcast(bpb[:], bp[:])
              Wlw_sb = sp.tile([8, G3], F32, tag="wlw")
              nc.sync.dma_start(Wlw_sb[:], Wlw[:])
              Wlb_sb = sp.tile([8, 1], F32, tag="wlb")
              nc.sync.dma_start(Wlb_sb[:], Wlb[:])
              comb = rp.tile([8, G3 + 1], F32)
              nc.vector.tensor_tensor(comb[:, :G3], Wlw_sb[:], gpb[:], OP.mult)
              vb = sp.tile([8, G3], F32, tag="vb")
              nc.vector.tensor_tensor(vb[:], Wlw_sb[:], bpb[:], OP.mult)
              vbr = sp.tile([8, 1], F32, tag="vbr")
              nc.vector.tensor_reduce(vbr[:], vb[:], mybir.AxisListType.X, OP.add)
              nc.vector.tensor_tensor(comb[:, G3:], Wlb_sb[:], vbr[:], OP.add)
              combb = rp.tile([8, G3 + 1], BF16)
              nc.scalar.copy(combb[:], comb[:])

              y_sb = rp.tile([128, c.p_chunks], F32)
              for g in range(c.p_chunks):
                  pw = psR.tile([128, G3 + 1], F32, tag="pw")
                  nc.tensor.matmul(pw[:], ea3_sb[:, ts(g, 128)], combb[:],
                                   start=True, stop=True)
                  scr = sp.tile([128, G3 + 1], F32, tag="scr")
                  nc.vector.scalar_tensor_tensor(
                      scr[:],
                      _ap(yh[:], [[c.p_chunks * PIT, 128], [1, G3 + 1]],
                          off=g * PIT),
                      1.0, pw[:], OP.mult, OP.mult,
                      accum_out=y_sb[:, ds(g, 1)])
              nc.sync.dma_start(y[:], y_sb[:])

    _insert_library_loads(nc)
    mybir.codegen_inst_isa_subclasses(nc)
    _legalize_waits(nc)
    return nc


_NC_CACHE = {}


def _get_nc(cfg: Cfg):
    nc = _NC_CACHE.get(cfg.key)
    if nc is None:
        nc = build_nc(cfg)
        _NC_CACHE[cfg.key] = nc
    return nc


def _wrap16(a):
    """int token array [M] -> [128, M//16] int16 gather-index layout."""
    m = a.shape[0]
    return np.ascontiguousarray(
        np.tile(a.astype(np.int16).reshape(m // 16, 16).T, (8, 1)))


def preprocess(inputs, min_ch=None):
    """Full-problem inputs -> (cfg, list of 8 per-core input maps)."""
    x = np.asarray(inputs["x"], np.float32)
    edge_attr = np.asarray(inputs["edge_attr"], np.float32)
    edge_attr3 = np.asarray(inputs["edge_attr3"], np.float32)
    edge_index = np.asarray(inputs["edge_index"], np.int64)
    edge_index3 = np.asarray(inputs["edge_index3"], np.int64)
    n, e, p = x.shape[0], edge_index.shape[1], edge_index3.shape[1]
    n_loc = n // N_CORES
    tile_nodes = min(TILE, n_loc)

    src, dst = edge_index[0], edge_index[1]
    deg = np.maximum(np.bincount(dst, minlength=n), 1).astype(np.float32)
    rdeg = (1.0 / deg).astype(np.float32)
    order = np.argsort(dst, kind="stable")

    # node-table row remap: local row j=g*128+p stored at p*NG+g so the
    # per-iteration cc_in DMA is contiguous per partition
    NG = n_loc // 128
    jloc = np.arange(n_loc)
    locperm = (jloc % 128) * NG + jloc // 128

    def remap_glob(s):
        return (s // n_loc) * n_loc + locperm[s % n_loc]

    # tile boundaries
    dst_sorted = dst[order]
    n_tile_tot = n // tile_nodes
    bounds = np.searchsorted(dst_sorted, np.arange(n_tile_tot + 1) * tile_nodes)
    counts = np.diff(bounds)
    # chunk 0 of each tile is reserved for (up to 128) core-local-src
    # edges; the remaining edges start at slot 128, so a tile needs
    # 1 + ceil((cnt - l)/128) chunks.
    ch_per_tile = 1
    for gt in range(n_tile_tot):
        sel = order[bounds[gt]:bounds[gt + 1]]
        core = gt // (n_tile_tot // N_CORES)
        l = min(int(((src[sel] // n_loc) == core).sum()), 128)
        ch_per_tile = max(ch_per_tile,
                          1 + int(np.ceil((sel.shape[0] - l) / 128)))
    if min_ch is not None:
        ch_per_tile = max(ch_per_tile, min_ch)
    # pairs sharded by idx30's owning core (padded to a common length)
    own3 = edge_index3[0] // n_loc
    order3 = np.argsort(own3, kind="stable")
    counts3 = np.bincount(own3, minlength=N_CORES)
    p_loc_pad = int(np.ceil(max(counts3.max(), 128) / 128) * 128)
    cfg = Cfg(n, e, p, ch_per_tile, p_loc_pad)
    cfg.order3 = order3
    cfg.counts3 = counts3
    tiles_per_core = n_loc // tile_nodes

    Wn = np.asarray(inputs["Wn"], np.float32)
    bn = np.asarray(inputs["bn"], np.float32)
    We = np.asarray(inputs["We"], np.float32)
    be = np.asarray(inputs["be"], np.float32)
    W1 = np.asarray(inputs["W1"], np.float32)
    b1 = np.asarray(inputs["b1"], np.float32)
    W2 = np.asarray(inputs["W2"], np.float32)
    b2 = np.asarray(inputs["b2"], np.float32)
    conv_b = np.asarray(inputs["conv_b"], np.float32)
    Wih = np.asarray(inputs["Wih"], np.float32)
    Whh = np.asarray(inputs["Whh"], np.float32)
    bih = np.asarray(inputs["bih"], np.float32)
    bhh = np.asarray(inputs["bhh"], np.float32)
    Wlw = np.asarray(inputs["Wlw"], np.float32)
    Wlb = np.asarray(inputs["Wlb"], np.float32)
    gamma = np.asarray(inputs["gamma"], np.float32)
    beta = np.asarray(inputs["beta"], np.float32)

    xT = np.vstack([x.T, np.ones((1, n), np.float32)])
    Wn_ext = np.vstack([Wn, bn[None, :]])
    # WedgeT layout: per edge [o-major, i-minor]
    W2p = np.ascontiguousarray(
        W2.reshape(HID, DIM, DIM).transpose(0, 2, 1).reshape(HID, DIM * DIM))
    b2r = np.ascontiguousarray(b2.reshape(DIM, DIM))
    conv_bc = np.tile(conv_b[None, :], (128, 1))
    WihT = np.ascontiguousarray(Wih.T)
    WhhT = np.ascontiguousarray(Whh.T)
    bsum = bih + bhh
    b_r = bsum[0:DIM, None].copy()
    b_z = bsum[DIM:2 * DIM, None].copy()
    bihn = bih[2 * DIM:, None].copy()
    bhhn = bhh[2 * DIM:, None].copy()

    shared = dict(
        Wn_ext=Wn_ext, We_in=We, W1_in=W1,
        be_c=be[:, None], b1_c=b1[:, None],
        conv_bc=conv_bc, b2r_in=b2r,
        b_r=b_r, b_z=b_z, bihn=bihn, bhhn=bhhn,
        Wlw=Wlw, Wlb=Wlb.reshape(8, 1),
        gamma=gamma[None, :], beta=beta[None, :],
        epsv=np.concatenate([np.full(DIM, 4.0 * EPS, np.float32),
                             np.full(2 * DIM, EPS, np.float32)])[None, :],
    )
    shared = {k: np.ascontiguousarray(v, dtype=np.float32)
              for k, v in shared.items()}
    shared["W2p_in"] = W2p.astype(ml_dtypes.bfloat16)
    shared["WihT"] = WihT.astype(ml_dtypes.bfloat16)
    shared["WhhT"] = WhhT.astype(ml_dtypes.bfloat16)

    cum3 = np.concatenate([[0], np.cumsum(counts3)])
    CH = cfg.ch_per_tile
    in_maps = []
    for core in range(N_CORES):
        lidx_tok = np.zeros(tiles_per_core * 128, np.int64)
        gidx_tok = np.zeros(tiles_per_core * (CH - 1) * 128, np.int64)
        ea_rows = np.zeros((cfg.e_pad, edge_attr.shape[1]), np.float32)
        ohc = np.zeros((cfg.chunks, 128, TILE), np.float32)
        for t in range(tiles_per_core):
            gt = core * tiles_per_core + t
            sel = order[bounds[gt]:bounds[gt + 1]]
            # chunk 0 of the tile holds only core-local-src edges (<=128),
            # gathered from the core-local table before the AllGather lands
            is_loc = (src[sel] // n_loc) == core
            if is_loc.sum() > 128:
                is_loc[np.where(is_loc)[0][128:]] = False
            sel_l, sel_g = sel[is_loc], sel[~is_loc]
            nl_, ng_ = sel_l.shape[0], sel_g.shape[0]
            base = t * CH * 128
            lidx_tok[t * 128:t * 128 + nl_] = \
                locperm[src[sel_l] - core * n_loc]
            gidx_tok[t * (CH - 1) * 128:t * (CH - 1) * 128 + ng_] = \
                remap_glob(src[sel_g])
            ea_rows[base:base + nl_] = edge_attr[sel_l]
            ea_rows[base + 128:base + 128 + ng_] = edge_attr[sel_g]
            pos = np.arange(nl_)
            ohc[t * CH, pos, dst[sel_l] - gt * tile_nodes] = rdeg[dst[sel_l]]
            pos = np.arange(ng_)
            ohc[t * CH + 1 + pos // 128, pos % 128,
                dst[sel_g] - gt * tile_nodes] = rdeg[dst[sel_g]]
        ohp_core = np.ascontiguousarray(
            ohc.transpose(1, 0, 2).reshape(128, cfg.chunks * TILE)
        ).astype(ml_dtypes.bfloat16)
        sel3 = order3[cum3[core]:cum3[core + 1]]
        cnt3 = sel3.shape[0]
        i30l = np.zeros(p_loc_pad, np.int64)
        i30l[:cnt3] = locperm[edge_index3[0, sel3] - core * n_loc]
        i31g = np.zeros(p_loc_pad, np.int64)
        i31g[:cnt3] = remap_glob(edge_index3[1, sel3])
        ea3c = np.zeros((p_loc_pad, 8), np.float32)
        ea3c[:cnt3] = edge_attr3[sel3]
        pmv = np.zeros(p_loc_pad, np.float32)
        pmv[:cnt3] = 1.0
        m = dict(shared)
        m.update(
            xTl=np.ascontiguousarray(xT[:, core * n_loc:(core + 1) * n_loc]),
            eaT=np.ascontiguousarray(ea_rows.T),
            gidxl=_wrap16(lidx_tok),
            gidxg=_wrap16(gidx_tok),
            ohp=ohp_core,
            idx30=_wrap16(i30l),
            idx31=_wrap16(i31g),
            ea3T=np.ascontiguousarray(ea3c.T.astype(ml_dtypes.bfloat16)),
            pmask_in=np.ascontiguousarray(
                pmv.reshape(cfg.p_chunks, 128).T.astype(ml_dtypes.bfloat16)),
        )
        in_maps.append(m)
    return cfg, in_maps


def postprocess(cfg: Cfg, results):
    out = np.empty(cfg.p, np.float32)
    pos = 0
    for core in range(N_CORES):
        cnt = int(cfg.counts3[core])
        yc = results[core]["y"]            # [128, p_chunks]; pair j = g*128+p
        out[cfg.order3[pos:pos + cnt]] = yc.T.reshape(-1)[:cnt]
        pos += cnt
    return out


def kernel(**inputs):
    cfg, in_maps = preprocess(inputs)
    nc = _get_nc(cfg)
    res = bass_utils.run_bass_kernel_spmd(nc, in_maps,
                                          core_ids=list(range(N_CORES)))
    return postprocess(cfg, res.results)


# revision 63
# speedup vs baseline: 1.1661x; 1.1661x over previous
"""Trainium2 Bass kernel for nn_Net_int_12421045420311 (GNN message passing).

Model (see problem reference):
  out = relu(x @ Wn + bn)                         [N, 64]
  ea  = relu(edge_attr @ We + be)                 [E, 12]
  Wedge = (relu(ea @ W1 + b1) @ W2 + b2)          [E, 64, 64]
  3x: msg_e = out[src_e] @ Wedge_e ; agg = scatter_mean(msg, dst) ;
      h = GRU(relu(agg + conv_b), h) ; out = h
  pair readout over 65536 node pairs + train-mode batchnorm + linear head.

Distribution over 8 cores: edges sorted by dst and sharded by dst range
(N/8 nodes per core) so every core owns complete scatter sums for its
nodes; the node table is AllGathered (bf16) between iterations; pairs
are sharded for the readout and batchnorm stats are AllReduced.

Device techniques (v2):
 - scatter-mean one-hots (with 1/deg folded in) precomputed on HOST as
   bf16 and kept resident in SBUF -- no per-iteration GPSIMD build.
 - Wedge stored TRANSPOSED per edge ([o-major, i-minor]) with the edge
   MLP bias b2 folded in (K=1 ones-row matmul in phase A), so the
   per-edge contraction msg_e = sum_i a_e[i] * W_e[:, i] runs as a
   dense bf16 2x-mode tensor_tensor + a 2-level 2x add tree + a 16-way
   dense reduce on DVE.
 - phase A (per-edge weight matrices) fused into iteration 0: each
   Wedge chunk is consumed from SBUF and simultaneously written to
   DRAM (bf16) for iterations 1-2 to stream back.
 - node tables bf16 end-to-end (gathers, AllGathers); GRU state f32.
 - gathers via the GPSIMD dma_gather extended instruction.
"""

from contextlib import ExitStack

import numpy as np
import ml_dtypes

import concourse.bass as bass
import concourse.mybir as mybir
import concourse.tile as tile
import bass_rust as _bass_rust
from concourse import bass_utils
from concourse.bass import ts, ds
from concourse.library_config import all_libraries, standard
from concourse.masks import make_identity

F32 = mybir.dt.float32
BF16 = mybir.dt.bfloat16
I16 = mybir.dt.int16
AF = mybir.ActivationFunctionType
OP = mybir.AluOpType

N_CORES = 8
DIM = 64
GATHER_PIECE = 1024   # max idxs per dma_gather (descriptor-ring capacity)
HID = 128
EDIM = 12
EPS = 1e-5
TILE = 512            # nodes per scatter tile


class Cfg:
    def __init__(self, n_nodes, n_edges, n_pairs, ch_per_tile, p_loc,
                 n_iters=3, readout=True):
        self.n = n_nodes
        self.e = n_edges
        self.p = n_pairs
        self.n_loc = n_nodes // N_CORES
        self.p_loc = p_loc          # padded pairs per core (idx30-sharded)
        self.tile_nodes = min(TILE, self.n_loc)
        self.n_tiles = self.n_loc // self.tile_nodes
        self.subt = self.tile_nodes // 128
        self.ch_per_tile = ch_per_tile
        self.chunks = self.n_tiles * ch_per_tile
        self.e_pad = self.chunks * 128
        self.p_chunks = self.p_loc // 128
        self.n_iters = n_iters
        self.readout = readout
        self.key = (n_nodes, n_edges, n_pairs, ch_per_tile, p_loc,
                    n_iters, readout)


def _ap(base, dims, off=0):
    return bass.AP(base.tensor, base.offset + off, [list(d) for d in dims])


def _legalize_waits(nc, keep=1, keep_extended=0):
    """Split multi-wait instructions into preceding single-wait NoOps.

    This walrus build's setupSyncWait accepts at most one sync wait per
    instruction, while Tile attaches one wait per producer semaphore.
    Waits execute on the engine sequencer in program order, so hoisting
    them onto NoOps preserves semantics.
    """
    n = 0
    ext = ("DMAGatherAnt", "DMAScatterAddAnt", "KVWritebackAnt",
           "PagedWritebackAnt")
    for f in nc.m.functions:
        for bb in f.blocks:
            out = []
            for ins in bb.instructions:
                si = ins.sync_info
                k = keep_extended if type(ins).__name__.removeprefix("Inst") in ext else keep
                if si is not None and si.on_wait is not None and len(si.on_wait) > k:
                    waits = list(si.on_wait)
                    for w in (waits[:-k] if k else waits):
                        nop = mybir.InstNoOp(name=f"WS-{n}", text_hint="waitsplit")
                        n += 1
                        nop.engine = ins.engine
                        nop.sync_info = mybir.SyncInfo(on_wait=[w], on_update=[])
                        nc.register_instruction(nop, overwrite=True)
                        out.append(nop)
                    ins.sync_info = mybir.SyncInfo(
                        on_wait=(waits[-k:] if k else []),
                        on_update=list(si.on_update))
                out.append(ins)
            bb.instructions = out
    return n


def _insert_library_loads(nc):
    """bacc.insert_library_loads equivalent for plain Bass: dma_gather &
    friends need the 'mlp' GPSIMD ucode library loaded."""
    mask = {}
    for lib in all_libraries:
        for t in lib.instructions:
            mask[t] = mask.get(t, 0) | (1 << lib.index)
    _bass_rust.insert_library_loads(nc, mask, len(all_libraries), standard.index)


def _gather_split(nc, out_tile, table, idx_sb, total):
    """dma_gather in <=GATHER_PIECE chunks (descriptor carveout is ~1024
    descs; one big gather would deadlock awaiting ring space)."""
    for off in range(0, total, GATHER_PIECE):
        w = min(GATHER_PIECE, total - off)
        nc.gpsimd.dma_gather(
            out_tile[:, off // 128:(off + w) // 128, :], table[:],
            idx_sb[:, off // 16:(off + w) // 16], w, w, DIM)


def build_nc(c: Cfg):
    nc = bass.Bass()
    WSQ = DIM * DIM
    G3 = 3 * DIM

    # ---------------- I/O ----------------
    xTl = nc.dram_tensor("xTl", [9, c.n_loc], F32, kind="ExternalInput")
    Wn_ext = nc.dram_tensor("Wn_ext", [9, DIM], F32, kind="ExternalInput")
    eaT = nc.dram_tensor("eaT", [19, c.e_pad], F32, kind="ExternalInput")
    We_in = nc.dram_tensor("We_in", [19, EDIM], F32, kind="ExternalInput")
    W1_in = nc.dram_tensor("W1_in", [EDIM, HID], F32, kind="ExternalInput")
    be_c = nc.dram_tensor("be_c", [EDIM, 1], F32, kind="ExternalInput")
    b1_c = nc.dram_tensor("b1_c", [HID, 1], F32, kind="ExternalInput")
    W2p_in = nc.dram_tensor("W2p_in", [HID, WSQ], BF16, kind="ExternalInput")
    b2r_in = nc.dram_tensor("b2r_in", [DIM, DIM], F32, kind="ExternalInput")
    ohp = nc.dram_tensor("ohp", [128, c.chunks * TILE], BF16,
                         kind="ExternalInput")
    conv_bc = nc.dram_tensor("conv_bc", [128, DIM], F32, kind="ExternalInput")
    # per-tile chunk 0 holds only core-local-src edges (gathered from
    # cc_in, overlapping the AllGather); the rest gather from cc_out
    gidxl = nc.dram_tensor("gidxl", [128, c.n_tiles * 8], I16,
                           kind="ExternalInput")
    gidxg = nc.dram_tensor(
        "gidxg", [128, c.n_tiles * (c.ch_per_tile - 1) * 8], I16,
        kind="ExternalInput")
    WihT = nc.dram_tensor("WihT", [DIM, 3 * DIM], BF16, kind="ExternalInput")
    WhhT = nc.dram_tensor("WhhT", [DIM, 3 * DIM], BF16, kind="ExternalInput")
    b_r = nc.dram_tensor("b_r", [DIM, 1], F32, kind="ExternalInput")
    b_z = nc.dram_tensor("b_z", [DIM, 1], F32, kind="ExternalInput")
    bihn = nc.dram_tensor("bihn", [DIM, 1], F32, kind="ExternalInput")
    bhhn = nc.dram_tensor("bhhn", [DIM, 1], F32, kind="ExternalInput")
    idx30 = nc.dram_tensor("idx30", [128, c.p_loc // 16], I16, kind="ExternalInput")
    idx31 = nc.dram_tensor("idx31", [128, c.p_loc // 16], I16, kind="ExternalInput")
    pmask_in = nc.dram_tensor("pmask_in", [128, c.p_chunks], BF16,
                              kind="ExternalInput")
    ea3T = nc.dram_tensor("ea3T", [8, c.p_loc], BF16, kind="ExternalInput")
    Wlw = nc.dram_tensor("Wlw", [8, G3], F32, kind="ExternalInput")
    Wlb = nc.dram_tensor("Wlb", [8, 1], F32, kind="ExternalInput")
    gamma = nc.dram_tensor("gamma", [1, G3], F32, kind="ExternalInput")
    beta = nc.dram_tensor("beta", [1, G3], F32, kind="ExternalInput")
    epsv = nc.dram_tensor("epsv", [1, G3], F32, kind="ExternalInput")
    y = nc.dram_tensor("y", [128, c.p_chunks], F32, kind="ExternalOutput")

    # internal DRAM. Node tables are [h | h@b2r] bf16 (256B rows: the min
    # dma_gather granule) -- the b2r half supplies the edge-MLP bias term
    # of the NNConv directly through the scatter matmul.
    wedge_d = nc.dram_tensor("wedge_d", [c.e_pad, WSQ], BF16)
    cc_in = [nc.dram_tensor(f"cc_in{i}", [c.n_loc, 2 * DIM], BF16)
             for i in range(c.n_iters + 1)]
    cc_out = [nc.dram_tensor(f"cc_out{i}", [c.n, 2 * DIM], BF16,
                             addr_space="Shared")
              for i in range(c.n_iters + 1)]
    st_in = nc.dram_tensor("st_in", [1, 2 * G3], F32)
    st_out = nc.dram_tensor("st_out", [1, 2 * G3], F32, addr_space="Shared")

    rgroups = [list(range(N_CORES))]

    with tile.TileContext(nc) as tc:
      with (
          tc.tile_pool(name="persist", bufs=1) as pp,
      ):
        # ------------- persistent small tensors -------------
        ident = pp.tile([128, 128], F32)
        make_identity(nc, ident[:])

        Wn_sb = pp.tile([9, DIM], F32)
        nc.sync.dma_start(Wn_sb[:], Wn_ext[:])
        convb_sb = pp.tile([128, DIM], F32)
        nc.sync.dma_start(convb_sb[:], conv_bc[:])
        b2r_sb = pp.tile([DIM, DIM], F32)
        nc.sync.dma_start(b2r_sb[:], b2r_in[:])
        WihT_sb = pp.tile([DIM, 3 * DIM], BF16)
        nc.sync.dma_start(WihT_sb[:], WihT[:])
        WhhT_sb = pp.tile([DIM, 3 * DIM], BF16)
        nc.sync.dma_start(WhhT_sb[:], WhhT[:])
        br_sb = pp.tile([DIM, 1], F32)
        nc.sync.dma_start(br_sb[:], b_r[:])
        bz_sb = pp.tile([DIM, 1], F32)
        nc.sync.dma_start(bz_sb[:], b_z[:])
        bihn_sb = pp.tile([DIM, 1], F32)
        nc.sync.dma_start(bihn_sb[:], bihn[:])
        bhhn_sb = pp.tile([DIM, 1], F32)
        nc.sync.dma_start(bhhn_sb[:], bhhn[:])
        gidxl_sb = pp.tile([128, c.n_tiles * 8], I16)
        gidxg_sb = pp.tile([128, c.n_tiles * (c.ch_per_tile - 1) * 8], I16)
        oh_sb = pp.tile([128, c.chunks, TILE], BF16)

        # wedge-source tensors live only through iteration 0
        wstk = ExitStack()
        wdp = wstk.enter_context(tc.tile_pool(name="wdat", bufs=1))
        W2p_sb = wdp.tile([HID, WSQ], BF16)
        h1T = wdp.tile([HID, c.e_pad], BF16)

        hT = pp.tile([DIM, c.n_loc], F32)       # node state (transposed)
        hTb = pp.tile([DIM, c.n_loc], BF16)     # bf16 copy for GRU matmuls

        def emit_rows_ag(idx, psT, rows):
            """hT -> [h | h@b2r] bf16 rows -> cc_in[idx] -> AllGather."""
            for g in range(c.n_loc // 128):
                ptg = psT.tile([128, 128], F32, tag="ptx")
                nc.tensor.transpose(ptg[:, :DIM], hT[:, ts(g, 128)],
                                    ident[:DIM, :DIM])
                nc.scalar.copy(rows[:, g, 0:DIM], ptg[:, :DIM])
                pb2 = psT.tile([128, DIM], F32, tag="pb2")
                nc.tensor.matmul(pb2[:], hT[:, ts(g, 128)], b2r_sb[:],
                                 start=True, stop=True)
                nc.scalar.copy(rows[:, g, DIM:2 * DIM], pb2[:])
            # rows stored partition-major (row = p*NG + g): contiguous DMA;
            # all gather indices are host-remapped to match.
            nc.sync.dma_start(
                cc_in[idx].rearrange("(p g) d -> p g d", p=128), rows[:])
            nc.gpsimd.collective_compute(
                "AllGather", OP.bypass, replica_groups=rgroups,
                ins=[cc_in[idx].ap().opt()], outs=[cc_out[idx].ap().opt()])

        # ------------- phase 0: initial node embeddings -------------
        # h0T = relu(Wn_ext.T @ x_extT_local); table built via AllGather
        with (
            tc.tile_pool(name="p0", bufs=2) as sp,
            tc.tile_pool(name="p0ps", bufs=2, space="PSUM") as ps2,
        ):
            xl = sp.tile([9, c.n_loc], F32, tag="xl")
            nc.sync.dma_start(xl[:], xTl[:])
            for j in range(0, c.n_loc, 512):
                w = min(512, c.n_loc - j)
                ph = ps2.tile([DIM, 512], F32, tag="p0h")
                nc.tensor.matmul(ph[:, :w], Wn_sb[:], xl[:, ds(j, w)],
                                 start=True, stop=True)
                nc.scalar.activation(hT[:, ds(j, w)], ph[:, :w], AF.Relu)
            rows0 = sp.tile([128, c.n_loc // 128, 2 * DIM], BF16, tag="r0")
            emit_rows_ag(0, ps2, rows0)

        # bulk loads issued after the phase-0 critical path
        nc.sync.dma_start(gidxl_sb[:], gidxl[:])
        nc.sync.dma_start(gidxg_sb[:], gidxg[:])
        nc.sync.dma_start(
            oh_sb[:], ohp.rearrange("p (c t) -> p c t", c=c.chunks))
        nc.sync.dma_start(W2p_sb[:], W2p_in[:])

        # ------------- phase A prep: edge MLP -> h1T (bf16) -------------
        with (
            tc.tile_pool(name="pa", bufs=3) as sp,
            tc.tile_pool(name="pah", bufs=1) as hp,
            tc.tile_pool(name="paps", bufs=2, space="PSUM") as psA,
        ):
            W1_sb = hp.tile([EDIM, HID], F32)
            nc.sync.dma_start(W1_sb[:], W1_in[:])
            We_sb = hp.tile([19, EDIM], F32)
            nc.sync.dma_start(We_sb[:], We_in[:])
            be_sb = hp.tile([EDIM, 1], F32)
            nc.sync.dma_start(be_sb[:], be_c[:])
            b1_sb = hp.tile([HID, 1], F32)
            nc.sync.dma_start(b1_sb[:], b1_c[:])
            ea_sb = hp.tile([EDIM, c.e_pad], F32)
            for j in range(0, c.e_pad, 512):
                w = min(512, c.e_pad - j)
                et = sp.tile([19, 512], F32, tag="et")
                nc.sync.dma_start(et[:, :w], eaT[:, ds(j, w)])
                pe = psA.tile([EDIM, 512], F32, tag="pe")
                nc.tensor.matmul(pe[:, :w], We_sb[:], et[:, :w],
                                 start=True, stop=True)
                nc.scalar.activation(ea_sb[:, ds(j, w)], pe[:, :w],
                                     AF.Relu, bias=be_sb[:])
            for j in range(0, c.e_pad, 512):
                w = min(512, c.e_pad - j)
                ph1 = psA.tile([HID, 512], F32, tag="ph1")
                nc.tensor.matmul(ph1[:, :w], W1_sb[:], ea_sb[:, ds(j, w)],
                                 start=True, stop=True)
                nc.scalar.activation(h1T[:, ds(j, w)], ph1[:, :w], AF.Relu,
                                     bias=b1_sb[:])

        # ------------- 3 message-passing iterations -------------
        # iteration 0 also computes Wedge (transposed, +b2) chunk by chunk,
        # consuming each chunk from SBUF and writing it to DRAM for the
        # later iterations to stream back.
        for it in range(c.n_iters):
            tab_prev = cc_out[it]
            with (
                tc.tile_pool(name=f"it{it}", bufs=1) as ip,
                tc.tile_pool(name=f"itw{it}", bufs=3) as wp,
                tc.tile_pool(name=f"itv{it}", bufs=1) as vp,
                tc.tile_pool(name=f"its{it}", bufs=2) as sp,
                tc.tile_pool(name=f"itpt{it}", bufs=1, space="PSUM") as psT,
                tc.tile_pool(name=f"itpm{it}", bufs=2, space="PSUM") as psM,
            ):
                nc.scalar.copy(hTb[:], hT[:])
                # gathered rows [a | a@b2r]; msgs later overwrite the a half
                A_sb = ip.tile([128, c.chunks, 2 * DIM], BF16)
                CH = c.ch_per_tile
                for t in range(c.n_tiles):
                    nc.gpsimd.dma_gather(
                        A_sb[:, t * CH:t * CH + 1, :], cc_in[it][:],
                        gidxl_sb[:, t * 8:(t + 1) * 8], 128, 128, 2 * DIM)
                rows_pt = (CH - 1) * 128
                for t in range(c.n_tiles):
                    for off in range(0, rows_pt, GATHER_PIECE):
                        w = min(GATHER_PIECE, rows_pt - off)
                        c0 = t * CH + 1 + off // 128
                        nc.gpsimd.dma_gather(
                            A_sb[:, c0:c0 + w // 128, :], tab_prev[:],
                            gidxg_sb[:, (t * rows_pt + off) // 16:
                                     (t * rows_pt + off + w) // 16],
                            w, w, 2 * DIM)
                mT = ip.tile([DIM, c.n_loc], BF16)

                with tc.tile_pool(name=f"itpw{it}", bufs=2,
                                  space="PSUM") as psW:
                    def do_chunks(ch, nch):
                        # process nch (1 or 2) adjacent edge chunks in one
                        # DVE chain to amortize per-instruction overhead
                        NW = nch * WSQ
                        wt = wp.tile([128, 2, WSQ], BF16, tag="wt")
                        if it == 0:
                            # WedgeT chunks = h1 @ W2p, bf16
                            for j in range(nch):
                                for q in range(4):
                                    pwq = psW.tile([128, WSQ // 4], F32,
                                                   tag="pw")
                                    for hh in range(2):
                                        nc.tensor.matmul(
                                            pwq[:, ts(hh, 512)],
                                            h1T[:, ts(ch + j, 128)],
                                            W2p_sb[:, ds(q * 1024 + hh * 512,
                                                         512)],
                                            start=True, stop=True)
                                    nc.scalar.copy(
                                        wt[:, j, ds(q * 1024, 1024)], pwq[:])
                            nc.sync.dma_start(
                                _ap(wedge_d.ap(),
                                    [[WSQ, 128], [128 * WSQ, nch], [1, WSQ]],
                                    off=ch * 128 * WSQ),
                                wt[:, :nch, :])
                        else:
                            nc.sync.dma_start(
                                wt[:, :nch, :],
                                _ap(wedge_d.ap(),
                                    [[WSQ, 128], [128 * WSQ, nch], [1, WSQ]],
                                    off=ch * 128 * WSQ))
                        # msg_e[o] = sum_i a_e[i] * WT_e[o, i] on DVE
                        AD = c.chunks * 2 * DIM
                        prod = vp.tile([128, 2, WSQ], BF16, tag="prod")
                        nc.vector.tensor_tensor(
                            _ap(prod[:], [[2 * WSQ, 128], [WSQ, nch],
                                          [DIM, DIM], [1, DIM]]),
                            _ap(wt[:], [[2 * WSQ, 128], [WSQ, nch],
                                        [DIM, DIM], [1, DIM]]),
                            _ap(A_sb[:], [[AD, 128], [2 * DIM, nch],
                                          [0, DIM], [1, DIM]],
                                off=ch * 2 * DIM),
                            OP.mult)
                        pr1 = vp.tile([128, 2, WSQ // 2], BF16, tag="pr1")
                        nc.vector.tensor_tensor(
                            _ap(pr1[:], [[WSQ, 128], [2048, nch],
                                         [32, DIM], [1, 32]]),
                            _ap(prod[:], [[2 * WSQ, 128], [WSQ, nch],
                                          [DIM, DIM], [1, 32]]),
                            _ap(prod[:], [[2 * WSQ, 128], [WSQ, nch],
                                          [DIM, DIM], [1, 32]], off=32),
                            OP.add)
                        pr2 = vp.tile([128, 2, WSQ // 4], BF16, tag="pr2")
                        nc.vector.tensor_tensor(
                            _ap(pr2[:], [[WSQ // 2, 128], [1024, nch],
                                         [16, DIM], [1, 16]]),
                            _ap(pr1[:], [[WSQ, 128], [2048, nch],
                                         [32, DIM], [1, 16]]),
                            _ap(pr1[:], [[WSQ, 128], [2048, nch],
                                         [32, DIM], [1, 16]], off=16),
                            OP.add)
                        pr3 = vp.tile([128, 2, WSQ // 8], BF16, tag="pr3")
                        nc.vector.tensor_tensor(
                            _ap(pr3[:], [[WSQ // 4, 128], [512, nch],
                                         [8, DIM], [1, 8]]),
                            _ap(pr2[:], [[WSQ // 2, 128], [1024, nch],
                                         [16, DIM], [1, 8]]),
                            _ap(pr2[:], [[WSQ // 2, 128], [1024, nch],
                                         [16, DIM], [1, 8]], off=8),
                            OP.add)
                        with nc.allow_low_precision(
                                reason="bf16 msg sums ok at 2e-2 tol"):
                            nc.vector.tensor_reduce(
                                _ap(A_sb[:], [[AD, 128], [2 * DIM, nch],
                                              [1, DIM]], off=ch * 2 * DIM),
                                _ap(pr3[:], [[WSQ // 4, 128], [512, nch],
                                             [8, DIM], [1, 8]]),
                                mybir.AxisListType.X, OP.add)

                    # local-src chunks first: their gather needs no AllGather
                    for t in range(c.n_tiles):
                        do_chunks(t * CH, 1)
                    for t in range(c.n_tiles):
                        for k in range(1, CH, 2):
                            do_chunks(t * CH + k, min(2, CH - k))
                        # scatter-mean for tile t via one-hot matmuls; the
                        # moving operand [msgs | a@b2r] yields the message
                        # sum and the NNConv b2 term in one pass.
                        pm = psM.tile([128, c.subt * 2 * DIM], F32, tag="pm")
                        for s in range(c.subt):
                            for k in range(c.ch_per_tile):
                                ch = t * c.ch_per_tile + k
                                nc.tensor.matmul(
                                    pm[:, ts(s, 2 * DIM)],
                                    oh_sb[:, ch, ds(s * 128, 128)],
                                    A_sb[:, ch, :], start=(k == 0),
                                    stop=(k == c.ch_per_tile - 1))
                        btmp = sp.tile([128, c.subt * DIM], F32, tag="btmp")
                        nc.scalar.copy(
                            btmp[:],
                            _ap(pm[:], [[c.subt * 2 * DIM, 128],
                                        [2 * DIM, c.subt], [1, DIM]],
                                off=DIM))
                        mrow = sp.tile([128, c.subt * DIM], F32, tag="mrow")
                        nc.vector.tensor_tensor(
                            _ap(mrow[:], [[c.subt * DIM, 128], [DIM, c.subt],
                                          [1, DIM]]),
                            _ap(pm[:], [[c.subt * 2 * DIM, 128],
                                        [2 * DIM, c.subt], [1, DIM]]),
                            btmp[:], OP.add)
                        nc.vector.tensor_tensor(
                            mrow[:], mrow[:],
                            _ap(convb_sb[:], [[DIM, 128], [0, c.subt], [1, DIM]]),
                            OP.add)
                        nc.scalar.activation(mrow[:], mrow[:], AF.Relu)
                        for s in range(c.subt):
                            ptm = psT.tile([128, 128], F32, tag="ptx")
                            nc.tensor.transpose(ptm[:DIM, :], mrow[:, ts(s, DIM)],
                                                ident[:])
                            nc.scalar.copy(
                                mT[:, ds(t * c.tile_nodes + s * 128, 128)],
                                ptm[:DIM, :])

                # ---- GRU (transposed space), 512-node pieces ----
                with tc.tile_pool(name=f"itpg{it}", bufs=1,
                                  space="PSUM") as psG:
                    for j in range(0, c.n_loc, 512):
                        w = min(512, c.n_loc - j)
                        pr = psG.tile([DIM, 512], F32, tag="pr")
                        pz = psG.tile([DIM, 512], F32, tag="pz")
                        pxn = psG.tile([DIM, 512], F32, tag="pxn")
                        phn = psG.tile([DIM, 512], F32, tag="phn")
                        nc.tensor.matmul(pr[:, :w], WihT_sb[:, 0:DIM],
                                         mT[:, ds(j, w)], start=True, stop=False)
                        nc.tensor.matmul(pr[:, :w], WhhT_sb[:, 0:DIM],
                                         hTb[:, ds(j, w)], start=False, stop=True)
                        nc.tensor.matmul(pz[:, :w], WihT_sb[:, DIM:2 * DIM],
                                         mT[:, ds(j, w)], start=True, stop=False)
                        nc.tensor.matmul(pz[:, :w], WhhT_sb[:, DIM:2 * DIM],
                                         hTb[:, ds(j, w)], start=False, stop=True)
                        nc.tensor.matmul(pxn[:, :w], WihT_sb[:, 2 * DIM:],
                                         mT[:, ds(j, w)], start=True, stop=True)
                        nc.tensor.matmul(phn[:, :w], WhhT_sb[:, 2 * DIM:],
                                         hTb[:, ds(j, w)], start=True, stop=True)
                        r_sb = sp.tile([DIM, 512], F32, tag="r")
                        nc.scalar.activation(r_sb[:, :w], pr[:, :w], AF.Sigmoid,
                                             bias=br_sb[:])
                        z_sb = sp.tile([DIM, 512], F32, tag="z")
                        nc.scalar.activation(z_sb[:, :w], pz[:, :w], AF.Sigmoid,
                                             bias=bz_sb[:])
                        ghn = sp.tile([DIM, 512], F32, tag="ghn")
                        nc.scalar.activation(ghn[:, :w], phn[:, :w], AF.Identity,
                                             bias=bhhn_sb[:])
                        nc.vector.tensor_tensor(ghn[:, :w], r_sb[:, :w], ghn[:, :w],
                                                OP.mult)
                        s_sb = sp.tile([DIM, 512], F32, tag="s")
                        nc.vector.tensor_tensor(s_sb[:, :w], pxn[:, :w], ghn[:, :w],
                                                OP.add)
                        n_sb = sp.tile([DIM, 512], F32, tag="n")
                        nc.scalar.activation(n_sb[:, :w], s_sb[:, :w], AF.Tanh,
                                             bias=bihn_sb[:])
                        d_sb = sp.tile([DIM, 512], F32, tag="d")
                        nc.vector.tensor_tensor(d_sb[:, :w], hT[:, ds(j, w)],
                                                n_sb[:, :w], OP.subtract)
                        nc.vector.tensor_tensor(d_sb[:, :w], z_sb[:, :w], d_sb[:, :w],
                                                OP.mult)
                        nc.vector.tensor_tensor(hT[:, ds(j, w)], n_sb[:, :w],
                                                d_sb[:, :w], OP.add)

                # ---- rows of new h -> cc_in[it+1]; AllGather ----
                rows = ip.tile([128, c.n_loc // 128, 2 * DIM], BF16)
                emit_rows_ag(it + 1, psT, rows)
            if it == 0:
                wstk.close()    # free W2p/h1T SBUF for later phases

        # ------------- readout -------------
        if not c.readout:
            dbg = pp.tile([128, 8], F32)
            nc.sync.dma_start(dbg[:, :1], cc_out[c.n_iters][:128, :1])
            nc.vector.tensor_copy(dbg[:, 1:2], dbg[:, :1])
            yz = pp.tile([128, c.p_chunks], F32)
            nc.vector.memset(yz[:], 0.0)
            nc.vector.tensor_tensor(yz[:, :1], yz[:, :1], dbg[:, 1:2], OP.add)
            nc.sync.dma_start(y[:], yz[:])
        if c.readout:
          with (
              tc.tile_pool(name="ro", bufs=1) as rp,
              tc.tile_pool(name="ros", bufs=2) as sp,
              tc.tile_pool(name="rops", bufs=2, space="PSUM") as psR,
              tc.tile_pool(name="rop1", bufs=1, space="PSUM") as psS,
          ):
              tab_fin = cc_out[c.n_iters]
              GP = GATHER_PIECE
              t0 = rp.tile([128, c.p_chunks, 2 * DIM], BF16)
              i30 = rp.tile([128, c.p_loc // 16], I16)
              nc.sync.dma_start(i30[:], idx30[:])
              t1 = rp.tile([128, c.p_chunks, 2 * DIM], BF16)
              i31 = rp.tile([128, c.p_loc // 16], I16)
              nc.sync.dma_start(i31[:], idx31[:])
              ea3_sb = rp.tile([8, c.p_loc], BF16)
              nc.sync.dma_start(ea3_sb[:], ea3T[:])
              pmask = rp.tile([128, c.p_chunks], BF16)
              nc.sync.dma_start(pmask[:], pmask_in[:])

              PIT = G3 + 1
              yh = rp.tile([128, c.p_chunks, PIT], BF16)  # [sum|prod|diff2|1]
              nc.vector.memset(yh[:, :, G3:G3 + 1], 1.0)
              pst = psS.tile([1, G3], F32, tag="ps_s")
              psq = psS.tile([1, G3], F32, tag="ps_q")
              # pairs are sharded by idx30's owner core, so t0 gathers hit
              # the core-local final rows (cc_in) and overlap the final
              # AllGather; only t1 needs the gathered table. yh and the
              # batchnorm stats are built per t1 piece.
              for off in range(0, c.p_loc, GP):
                  w = min(GP, c.p_loc - off)
                  nc.gpsimd.dma_gather(
                      t0[:, off // 128:(off + w) // 128, :],
                      cc_in[c.n_iters][:],
                      i30[:, off // 16:(off + w) // 16], w, w, 2 * DIM)
              for off in range(0, c.p_loc, GP):
                  w = min(GP, c.p_loc - off)
                  g0, gw = off // 128, w // 128
                  nc.gpsimd.dma_gather(
                      t1[:, g0:g0 + gw, :], tab_fin[:],
                      i31[:, off // 16:(off + w) // 16], w, w, 2 * DIM)
                  pcD = [[c.p_chunks * 2 * DIM, 128], [2 * DIM, gw], [1, DIM]]
                  yhD = [[c.p_chunks * PIT, 128], [PIT, gw], [1, DIM]]
                  pco = g0 * 2 * DIM
                  yho = g0 * PIT
                  nc.vector.tensor_tensor(
                      _ap(yh[:], yhD, off=yho), _ap(t0[:], pcD, off=pco),
                      _ap(t1[:], pcD, off=pco), OP.add)
                  nc.vector.tensor_tensor(
                      _ap(yh[:], yhD, off=yho + DIM), _ap(t0[:], pcD, off=pco),
                      _ap(t1[:], pcD, off=pco), OP.mult)
                  nc.vector.tensor_tensor(
                      _ap(yh[:], yhD, off=yho + 2 * DIM),
                      _ap(t0[:], pcD, off=pco),
                      _ap(t1[:], pcD, off=pco), OP.subtract)
                  nc.scalar.square(_ap(yh[:], yhD, off=yho + 2 * DIM),
                                   _ap(yh[:], yhD, off=yho + 2 * DIM))
                  for g in range(g0, g0 + gw):
                      ysq = sp.tile([128, G3], BF16, tag="ysq")
                      nc.scalar.square(
                          ysq[:],
                          _ap(yh[:], [[c.p_chunks * PIT, 128], [1, G3]],
                              off=g * PIT))
                      nc.tensor.matmul(
                          pst[:], pmask[:, ds(g, 1)],
                          _ap(yh[:], [[c.p_chunks * PIT, 128], [1, G3]],
                              off=g * PIT),
                          start=(g == 0), stop=(g == c.p_chunks - 1))
                      nc.tensor.matmul(
                          psq[:], pmask[:, ds(g, 1)], ysq[:],
                          start=(g == 0), stop=(g == c.p_chunks - 1))
              st_sb = sp.tile([1, 2 * G3], F32, tag="st")
              nc.vector.tensor_copy(st_sb[:, :G3], pst[:])
              nc.scalar.copy(st_sb[:, G3:], psq[:])
              nc.sync.dma_start(st_in[:], st_sb[:])
              nc.gpsimd.collective_compute(
                  "AllReduce", OP.add, replica_groups=rgroups,
                  ins=[st_in.ap().opt()], outs=[st_out.ap().opt()])
              stg = sp.tile([1, 2 * G3], F32, tag="stg")
              nc.sync.dma_start(stg[:], st_out[:])

              # mu, var, rstd (with one Newton step), g' = rstd*gamma,
              # b' = beta - mu*g'
              gam_sb = sp.tile([1, G3], F32, tag="gam")
              nc.sync.dma_start(gam_sb[:], gamma[:])
              bet_sb = sp.tile([1, G3], F32, tag="bet")
              nc.sync.dma_start(bet_sb[:], beta[:])
              mu = sp.tile([1, G3], F32, tag="mu")
              nc.vector.tensor_scalar_mul(mu[:], stg[:, :G3], 1.0 / c.p)
              var = sp.tile([1, G3], F32, tag="var")
              nc.vector.tensor_scalar_mul(var[:], stg[:, G3:], 1.0 / c.p)
              musq = sp.tile([1, G3], F32, tag="musq")
              nc.vector.tensor_tensor(musq[:], mu[:], mu[:], OP.mult)
              nc.vector.tensor_tensor(var[:], var[:], musq[:], OP.subtract)
              epsv_sb = sp.tile([1, G3], F32, tag="epsv")
              nc.sync.dma_start(epsv_sb[:], epsv[:])
              ve = sp.tile([1, G3], F32, tag="ve")
              nc.vector.tensor_tensor(ve[:], var[:], epsv_sb[:], OP.add)
              sq = sp.tile([1, G3], F32, tag="sq")
              nc.scalar.activation(sq[:], ve[:], AF.Sqrt)
              r0 = sp.tile([1, G3], F32, tag="r0")
              nc.vector.reciprocal(r0[:], sq[:])
              # Newton: r1 = r0*(1.5 - 0.5*(var+eps)*r0^2)
              t_ = sp.tile([1, G3], F32, tag="t_")
              nc.vector.tensor_tensor(t_[:], r0[:], r0[:], OP.mult)
              nc.vector.tensor_tensor(t_[:], t_[:], ve[:], OP.mult)
              nc.vector.tensor_scalar(t_[:], t_[:], -0.5, 1.5, OP.mult, OP.add)
              nc.vector.tensor_tensor(r0[:], r0[:], t_[:], OP.mult)
              gp = sp.tile([1, G3], F32, tag="gp")
              nc.vector.tensor_tensor(gp[:], r0[:], gam_sb[:], OP.mult)
              bp = sp.tile([1, G3], F32, tag="bp")
              nc.vector.tensor_tensor(bp[:], mu[:], gp[:], OP.mult)
              nc.vector.tensor_tensor(bp[:], bet_sb[:], bp[:], OP.subtract)

              gpb = sp.tile([8, G3], F32, tag="gpb")
              nc.gpsimd.partition_broadcast(gpb[:], gp[:])
              bpb = sp.tile([8, G3], F32, tag="bpb")
              nc.gpsimd.partition_broadcast(bpb[:], bp[:])
              Wlw_sb = sp.tile([8, G3], F32, tag="wlw")
              nc.sync.dma_start(Wlw_sb[:], Wlw[:])
              Wlb_sb = sp.tile([8, 1], F32, tag="wlb")
              nc.sync.dma_start(Wlb_sb[:], Wlb[:])
              comb = rp.tile([8, G3 + 1], F32)
              nc.vector.tensor_tensor(comb[:, :G3], Wlw_sb[:], gpb[:], OP.mult)
              vb = sp.tile([8, G3], F32, tag="vb")
              nc.vector.tensor_tensor(vb[:], Wlw_sb[:], bpb[:], OP.mult)
              vbr = sp.tile([8, 1], F32, tag="vbr")
              nc.vector.tensor_reduce(vbr[:], vb[:], mybir.AxisListType.X, OP.add)
              nc.vector.tensor_tensor(comb[:, G3:], Wlb_sb[:], vbr[:], OP.add)
              combb = rp.tile([8, G3 + 1], BF16)
              nc.scalar.copy(combb[:], comb[:])

              y_sb = rp.tile([128, c.p_chunks], F32)
              for g in range(c.p_chunks):
                  pw = psR.tile([128, G3 + 1], F32, tag="pw")
                  nc.tensor.matmul(pw[:], ea3_sb[:, ts(g, 128)], combb[:],
                                   start=True, stop=True)
                  scr = sp.tile([128, G3 + 1], F32, tag="scr")
                  nc.vector.scalar_tensor_tensor(
                      scr[:],
                      _ap(yh[:], [[c.p_chunks * PIT, 128], [1, G3 + 1]],
                          off=g * PIT),
                      1.0, pw[:], OP.mult, OP.mult,
                      accum_out=y_sb[:, ds(g, 1)])
              nc.sync.dma_start(y[:], y_sb[:])

    _insert_library_loads(nc)
    mybir.codegen_inst_isa_subclasses(nc)
    _legalize_waits(nc)
    return nc


_NC_CACHE = {}


def _get_nc(cfg: Cfg):
    nc = _NC_CACHE.get(cfg.key)
    if nc is None:
        nc = build_nc(cfg)
        _NC_CACHE[cfg.key] = nc
    return nc


def _wrap16(a):
    """int token array [M] -> [128, M//16] int16 gather-index layout."""
    m = a.shape[0]
    return np.ascontiguousarray(
        np.tile(a.astype(np.int16).reshape(m // 16, 16).T, (8, 1)))


def preprocess(inputs, min_ch=None):
    """Full-problem inputs -> (cfg, list of 8 per-core input maps)."""
    x = np.asarray(inputs["x"], np.float32)
    edge_attr = np.asarray(inputs["edge_attr"], np.float32)
    edge_attr3 = np.asarray(inputs["edge_attr3"], np.float32)
    edge_index = np.asarray(inputs["edge_index"], np.int64)
    edge_index3 = np.asarray(inputs["edge_index3"], np.int64)
    n, e, p = x.shape[0], edge_index.shape[1], edge_index3.shape[1]
    n_loc = n // N_CORES
    tile_nodes = min(TILE, n_loc)

    src, dst = edge_index[0], edge_index[1]
    deg = np.maximum(np.bincount(dst, minlength=n), 1).astype(np.float32)
    rdeg = (1.0 / deg).astype(np.float32)
    order = np.argsort(dst, kind="stable")

    # node-table row remap: local row j=g*128+p stored at p*NG+g so the
    # per-iteration cc_in DMA is contiguous per partition
    NG = n_loc // 128
    jloc = np.arange(n_loc)
    locperm = (jloc % 128) * NG + jloc // 128

    def remap_glob(s):
        return (s // n_loc) * n_loc + locperm[s % n_loc]

    # tile boundaries
    dst_sorted = dst[order]
    n_tile_tot = n // tile_nodes
    bounds = np.searchsorted(dst_sorted, np.arange(n_tile_tot + 1) * tile_nodes)
    counts = np.diff(bounds)
    # chunk 0 of each tile is reserved for (up to 128) core-local-src
    # edges; the remaining edges start at slot 128, so a tile needs
    # 1 + ceil((cnt - l)/128) chunks.
    ch_per_tile = 1
    for gt in range(n_tile_tot):
        sel = order[bounds[gt]:bounds[gt + 1]]
        core = gt // (n_tile_tot // N_CORES)
        l = min(int(((src[sel] // n_loc) == core).sum()), 128)
        ch_per_tile = max(ch_per_tile,
                          1 + int(np.ceil((sel.shape[0] - l) / 128)))
    if min_ch is not None:
        ch_per_tile = max(ch_per_tile, min_ch)
    # pairs sharded by idx30's owning core (padded to a common length)
    own3 = edge_index3[0] // n_loc
    order3 = np.argsort(own3, kind="stable")
    counts3 = np.bincount(own3, minlength=N_CORES)
    p_loc_pad = int(np.ceil(max(counts3.max(), 128) / 128) * 128)
    cfg = Cfg(n, e, p, ch_per_tile, p_loc_pad)
    cfg.order3 = order3
    cfg.counts3 = counts3
    tiles_per_core = n_loc // tile_nodes

    Wn = np.asarray(inputs["Wn"], np.float32)
    bn = np.asarray(inputs["bn"], np.float32)
    We = np.asarray(inputs["We"], np.float32)
    be = np.asarray(inputs["be"], np.float32)
    W1 = np.asarray(inputs["W1"], np.float32)
    b1 = np.asarray(inputs["b1"], np.float32)
    W2 = np.asarray(inputs["W2"], np.float32)
    b2 = np.asarray(inputs["b2"], np.float32)
    conv_b = np.asarray(inputs["conv_b"], np.float32)
    Wih = np.asarray(inputs["Wih"], np.float32)
    Whh = np.asarray(inputs["Whh"], np.float32)
    bih = np.asarray(inputs["bih"], np.float32)
    bhh = np.asarray(inputs["bhh"], np.float32)
    Wlw = np.asarray(inputs["Wlw"], np.float32)
    Wlb = np.asarray(inputs["Wlb"], np.float32)
    gamma = np.asarray(inputs["gamma"], np.float32)
    beta = np.asarray(inputs["beta"], np.float32)

    xT = np.vstack([x.T, np.ones((1, n), np.float32)])
    Wn_ext = np.vstack([Wn, bn[None, :]])
    # WedgeT layout: per edge [o-major, i-minor]
    W2p = np.ascontiguousarray(
        W2.reshape(HID, DIM, DIM).transpose(0, 2, 1).reshape(HID, DIM * DIM))
    b2r = np.ascontiguousarray(b2.reshape(DIM, DIM))
    conv_bc = np.tile(conv_b[None, :], (128, 1))
    WihT = np.ascontiguousarray(Wih.T)
    WhhT = np.ascontiguousarray(Whh.T)
    bsum = bih + bhh
    b_r = bsum[0:DIM, None].copy()
    b_z = bsum[DIM:2 * DIM, None].copy()
    bihn = bih[2 * DIM:, None].copy()
    bhhn = bhh[2 * DIM:, None].copy()

    shared = dict(
        Wn_ext=Wn_ext, We_in=We, W1_in=W1,
        be_c=be[:, None], b1_c=b1[:, None],
        conv_bc=conv_bc, b2r_in=b2r,
        b_r=b_r, b_z=b_z, bihn=bihn, bhhn=bhhn,
        Wlw=Wlw, Wlb=Wlb.reshape(8, 1),
        gamma=gamma[None, :], beta=beta[None, :],
        epsv=np.concatenate([np.full(DIM, 4.0 * EPS, np.float32),
                             np.full(2 * DIM, EPS, np.float32)])[None, :],
    )
    shared = {k: np.ascontiguousarray(v, dtype=np.float32)
              for k, v in shared.items()}
    shared["W2p_in"] = W2p.astype(ml_dtypes.bfloat16)
    shared["WihT"] = WihT.astype(ml_dtypes.bfloat16)
    shared["WhhT"] = WhhT.astype(ml_dtypes.bfloat16)

    cum3 = np.concatenate([[0], np.cumsum(counts3)])
    CH = cfg.ch_per_tile
    in_maps = []
    for core in range(N_CORES):
        lidx_tok = np.zeros(tiles_per_core * 128, np.int64)
        gidx_tok = np.zeros(tiles_per_core * (CH - 1) * 128, np.int64)
        ea_rows = np.zeros((cfg.e_pad, edge_attr.shape[1]), np.float32)
        ohc = np.zeros((cfg.chunks, 128, TILE), np.float32)
        for t in range(tiles_per_core):
            gt = core * tiles_per_core + t
            sel = order[bounds[gt]:bounds[gt + 1]]
            # chunk 0 of the tile holds only core-local-src edges (<=128),
            # gathered from the core-local table before the AllGather lands
            is_loc = (src[sel] // n_loc) == core
            if is_loc.sum() > 128:
                is_loc[np.where(is_loc)[0][128:]] = False
            sel_l, sel_g = sel[is_loc], sel[~is_loc]
            nl_, ng_ = sel_l.shape[0], sel_g.shape[0]
            base = t * CH * 128
            lidx_tok[t * 128:t * 128 + nl_] = \
                locperm[src[sel_l] - core * n_loc]
            gidx_tok[t * (CH - 1) * 128:t * (CH - 1) * 128 + ng_] = \
                remap_glob(src[sel_g])
            ea_rows[base:base + nl_] = edge_attr[sel_l]
            ea_rows[base + 128:base + 128 + ng_] = edge_attr[sel_g]
            pos = np.arange(nl_)
            ohc[t * CH, pos, dst[sel_l] - gt * tile_nodes] = rdeg[dst[sel_l]]
            pos = np.arange(ng_)
            ohc[t * CH + 1 + pos // 128, pos % 128,
                dst[sel_g] - gt * tile_nodes] = rdeg[dst[sel_g]]
        ohp_core = np.ascontiguousarray(
            ohc.transpose(1, 0, 2).reshape(128, cfg.chunks * TILE)
        ).astype(ml_dtypes.bfloat16)
        sel3 = order3[cum3[core]:cum3[core + 1]]
        cnt3 = sel3.shape[0]
        i30l = np.zeros(p_loc_pad, np.int64)
        i30l[:cnt3] = locperm[edge_index3[0, sel3] - core * n_loc]
        i31g = np.zeros(p_loc_pad, np.int64)
        i31g[:cnt3] = remap_glob(edge_index3[1, sel3])
        ea3c = np.zeros((p_loc_pad, 8), np.float32)
        ea3c[:cnt3] = edge_attr3[sel3]
        pmv = np.zeros(p_loc_pad, np.float32)
        pmv[:cnt3] = 1.0
        m = dict(shared)
        m.update(
            xTl=np.ascontiguousarray(xT[:, core * n_loc:(core + 1) * n_loc]),
            eaT=np.ascontiguousarray(ea_rows.T),
            gidxl=_wrap16(lidx_tok),
            gidxg=_wrap16(gidx_tok),
            ohp=ohp_core,
            idx30=_wrap16(i30l),
            idx31=_wrap16(i31g),
            ea3T=np.ascontiguousarray(ea3c.T.astype(ml_dtypes.bfloat16)),
            pmask_in=np.ascontiguousarray(
                pmv.reshape(cfg.p_chunks, 128).T.astype(ml_dtypes.bfloat16)),
        )
        in_maps.append(m)
    return cfg, in_maps


def postprocess(cfg: Cfg, results):
    out = np.empty(cfg.p, np.float32)
    pos = 0
    for core in range(N_CORES):
        cnt = int(cfg.counts3[core])
        yc = results[core]["y"]            # [128, p_chunks]; pair j = g*128+p
        out[cfg.order3[pos:pos + cnt]] = yc.T.reshape(-1)[:cnt]
        pos += cnt
    return out


def kernel(**inputs):
    cfg, in_maps = preprocess(inputs)
    nc = _get_nc(cfg)
    res = bass_utils.run_bass_kernel_spmd(nc, in_maps,
                                          core_ids=list(range(N_CORES)))
    return postprocess(cfg, res.results)
